# revision 6
# baseline (speedup 1.0000x reference)
"""MultiLabelContrastiveFocalLoss on 8 Trainium2 NeuronCores — v2.

Strategy (vs v1): host-side dtype casting slashes HBM traffic 3x, and the
Gram matmul runs in fp8 DoubleRow mode (2x-4x the bf16 rate) using the
shift trick th = tanh(x/2) = 2*(sigmoid(x)-0.5) with an exact rank-1
correction, validated to ~2.4e-5 rel err in numpy.

Math
----
loss = mean(focal) + (u2 - p2 - m2 + d)/D,  D = B*(B-1)
  focal: -ALPHA*(s^2*ln s + s^2*x*t) summed, s = sigmoid(-x)
  m2 = ||T^T P||_F^2 with P = (TH + 1)/2, TH = tanh(x/2):
     = 0.25*||T^T TH||^2 + 0.5*sum_l u_t[l]*rowsum(T^T TH)[l] + 128*||u_t||^2
  d  = sum_i rowT2_i*rowP2_i = WS * sum_l ((rowP2/WS)^T T)[l]
  u2 = ||colsum P||^2 = sum_m (0.5*colsum(TH)[m] + B/2)^2

Sharding (8 cores, SPMD, col-split — no cross-core communication)
-----------------------------------------------------------------
Core c (r=c//4, q=c%4): x-cols = quarter q (bf16, host-cast), t-cols =
parity-r half (fp8, host-cast, exact for 0/1). Each core computes the
[1024, 512] block of T^T TH with full k=4096, plus focal on its matching
256-col block. Per-core DMA: 8 MiB (was 24 MiB).
"""

import numpy as np
import ml_dtypes

import concourse.bacc as bacc
import concourse.bass as bass  # noqa: F401
import concourse.mybir as mybir
import concourse.tile as tile
from concourse.bass_utils import run_bass_kernel_spmd

mm = mybir.dt
AF = mybir.ActivationFunctionType
ALU = mybir.AluOpType
DR = mybir.MatmulPerfMode.DoubleRow

B, L = 4096, 2048
ALPHA = 0.25
N_CORES = 8
XC = L // 4            # 512  x-cols per core
TC = L // 2            # 1024 t-cols per core
FC = 256               # focal cols per core
NB = 4                 # k batches
KS = 8                 # k-subtiles (of 128 rows) per batch
WS = 1024.0            # scale for the w column of the aux matmul

_CACHE: dict = {}


def build_nc(loop_n=None):
    nc = bacc.Bacc("TRN2", target_bir_lowering=False, debug=False,
                   num_devices=N_CORES)
    xq_ext = nc.dram_tensor("xq", [B, XC], mm.bfloat16, kind="ExternalInput")
    th_ext = nc.dram_tensor("th", [B, TC], mm.float8e4, kind="ExternalInput")
    out_ext = nc.dram_tensor("out", [1, 8], mm.float32, kind="ExternalOutput")

    xq_t = xq_ext.ap().rearrange("(b s p) n -> b p s n", p=128, s=KS)
    th_t = th_ext.ap().rearrange("(b s p) n -> b p s n", p=128, s=KS)

    with tile.TileContext(nc) as tc:
        with (
            tc.tile_pool(name="tb", bufs=NB) as tb_pool,
            tc.tile_pool(name="xb", bufs=NB) as xb_pool,
            tc.tile_pool(name="ppb", bufs=NB) as ppb_pool,
            tc.tile_pool(name="foc", bufs=2) as foc_pool,
            tc.tile_pool(name="scr", bufs=2) as scr_pool,
            tc.tile_pool(name="stats", bufs=1) as stats_pool,
            tc.tile_pool(name="ps", bufs=8, space="PSUM") as ps_pool,
        ):
            def emit_body():
                rth = stats_pool.tile([128, NB * KS], mm.float32, tag="rth")
                rth2 = stats_pool.tile([128, NB * KS], mm.float32, tag="rth2")
                fst = stats_pool.tile([128, NB], mm.float32, tag="fst")
                f2st = stats_pool.tile([128, NB], mm.float32, tag="f2st")
                m2st = stats_pool.tile([128, 8], mm.float32, tag="m2st")
                stats2 = stats_pool.tile([128, 4], mm.float32, tag="stats2")
                ones_f32 = stats_pool.tile([128, 1], mm.float32, tag="onesf")
                nc.vector.memset(ones_f32[:], 1.0)
                lhTG = stats_pool.tile([128, NB * KS, 32], mm.float8e4,
                                       tag="lhTG")
                nc.vector.memset(lhTG[:], 0.0)
                nc.vector.memset(lhTG[:, :, 0:1], 1.0)
                osb = stats_pool.tile([1, 8], mm.float32, tag="osb")

                psA = [ps_pool.tile([128, XC], mm.float32, tag="bank",
                                    name=f"psA{m}") for m in range(8)]

                tb = [None] * NB
                ppb = [None] * NB
                for b in range(NB):
                    tb[b] = tb_pool.tile([128, KS, TC], mm.float8e4,
                                         name=f"tb{b}", tag="tb")
                    nc.sync.dma_start(out=tb[b][:], in_=th_t[b])
                    xb = xb_pool.tile([128, KS, XC], mm.bfloat16,
                                      name=f"xb{b}", tag="xb")
                    nc.sync.dma_start(out=xb[:], in_=xq_t[b])

                    # th = tanh(x/2) = 2p - 1, with per-ksub row accumulation
                    ppb[b] = ppb_pool.tile([128, KS, XC], mm.float8e4,
                                           name=f"ppb{b}", tag="ppb")
                    for s in range(KS):
                        nc.scalar.activation(
                            ppb[b][:, s, :], xb[:, s, :], AF.Tanh, scale=0.5,
                            accum_out=rth[:, b * KS + s:b * KS + s + 1])
                        scr = scr_pool.tile([128, XC], mm.bfloat16, tag="sq")
                        nc.vector.scalar_tensor_tensor(
                            out=scr[:], in0=ppb[b][:, s, :], scalar=1.0,
                            in1=ppb[b][:, s, :], op0=ALU.mult, op1=ALU.mult,
                            accum_out=rth2[:, b * KS + s:b * KS + s + 1])

                    # ---- focal on cols [0:FC] ----
                    sfo = foc_pool.tile([128, KS, FC], mm.bfloat16, tag="sfo")
                    nc.scalar.activation(sfo[:], xb[:, :, 0:FC], AF.Sigmoid,
                                         scale=-1.0)
                    s2 = foc_pool.tile([128, KS, FC], mm.bfloat16, tag="s2")
                    nc.gpsimd.tensor_tensor(out=s2[:], in0=sfo[:],
                                            in1=sfo[:], op=ALU.mult)
                    lns = foc_pool.tile([128, KS, FC], mm.bfloat16, tag="lns")
                    nc.scalar.activation(lns[:], sfo[:], AF.Ln)
                    tfo = foc_pool.tile([128, KS, FC], mm.bfloat16, tag="tfo")
                    nc.gpsimd.tensor_scalar(
                        out=tfo[:], in0=tb[b][:, :, 0:FC], scalar1=1.0,
                        scalar2=0.0, op0=ALU.mult, op1=ALU.add)
                    sx = foc_pool.tile([128, KS, FC], mm.bfloat16, tag="sx")
                    nc.vector.tensor_tensor(out=sx[:], in0=s2[:],
                                            in1=xb[:, :, 0:FC], op=ALU.mult)
                    f1s = foc_pool.tile([128, KS, FC], mm.bfloat16, tag="f1s")
                    nc.vector.scalar_tensor_tensor(
                        out=f1s[:], in0=s2[:], scalar=1.0, in1=lns[:],
                        op0=ALU.mult, op1=ALU.mult,
                        accum_out=fst[:, b:b + 1])
                    f2s = foc_pool.tile([128, KS, FC], mm.bfloat16, tag="f2s")
                    nc.vector.scalar_tensor_tensor(
                        out=f2s[:], in0=sx[:], scalar=1.0, in1=tfo[:],
                        op0=ALU.mult, op1=ALU.mult,
                        accum_out=f2st[:, b:b + 1])

                    # ---- main Gram matmul: psA[m] += th-block^T @ TH ----
                    for m in range(8):
                        for j in range(KS // 2):
                            nc.tensor.matmul(
                                psA[m][:],
                                tb[b][:, 2 * j:2 * j + 2,
                                      128 * m:128 * (m + 1)],
                                ppb[b][:, 2 * j:2 * j + 2, :],
                                start=(b == 0 and j == 0),
                                stop=(b == NB - 1 and j == KS // 2 - 1),
                                perf_mode=DR)

                # ---- m2 squares (frees psA banks afterwards) ----
                for m in range(8):
                    scrm = scr_pool.tile([128, XC], mm.bfloat16, tag="sq")
                    nc.scalar.activation(scrm[:], psA[m][:], AF.Square,
                                         accum_out=m2st[:, m:m + 1])

                # ---- build aux lhsT cols: w8 = w/WS, rp8 = rowTH ----
                wtmp = stats_pool.tile([128, NB * KS], mm.float32, tag="wtmp")
                nc.vector.scalar_tensor_tensor(
                    out=wtmp[:], in0=rth2[:], scalar=0.25, in1=rth[:],
                    op0=ALU.mult, op1=ALU.bypass)
                # wtmp = 0.25*rth2 ; then w = wtmp + 0.5*rth + 128
                w2 = stats_pool.tile([128, NB * KS], mm.float32, tag="w2")
                nc.vector.scalar_tensor_tensor(
                    out=w2[:], in0=rth[:], scalar=0.5, in1=wtmp[:],
                    op0=ALU.mult, op1=ALU.add)
                nc.vector.tensor_scalar(
                    out=lhTG[:, :, 1], in0=w2[:], scalar1=1.0 / WS,
                    scalar2=128.0 / WS, op0=ALU.mult, op1=ALU.add)
                nc.vector.tensor_scalar(
                    out=lhTG[:, :, 2], in0=rth[:], scalar1=1.0,
                    scalar2=0.0, op0=ALU.mult, op1=ALU.add)
                # p2 = sum(w) ; w = w2 + 128
                scrp = scr_pool.tile([128, NB * KS], mm.float32, tag="sw")
                nc.vector.tensor_scalar(
                    out=scrp[:], in0=w2[:], scalar1=1.0, scalar2=128.0,
                    op0=ALU.mult, op1=ALU.add, accum_out=stats2[:, 2:3])

                # ---- aux matmuls: [ones|w8|rp8]^T T  and  ones^T TH ----
                psT = [ps_pool.tile([32, XC], mm.float32, tag="bank",
                                    name=f"psT{nn}") for nn in range(2)]
                psUP = ps_pool.tile([32, XC], mm.float32, tag="bank")
                for j in range(2 * NB * KS // 4):   # 16 DR steps over k
                    b, jj = j // (KS // 2), j % (KS // 2)
                    for nn in range(2):
                        nc.tensor.matmul(
                            psT[nn][:],
                            lhTG[:, 2 * jj + b * KS:2 * jj + b * KS + 2, :],
                            tb[b][:, 2 * jj:2 * jj + 2,
                                  XC * nn:XC * (nn + 1)],
                            start=(j == 0), stop=(j == 15), perf_mode=DR)
                    nc.tensor.matmul(
                        psUP[:],
                        lhTG[:, 2 * jj + b * KS:2 * jj + b * KS + 2, :],
                        ppb[b][:, 2 * jj:2 * jj + 2, :],
                        start=(j == 0), stop=(j == 15), perf_mode=DR)

                # ---- row-scalar stats from aux outputs ----
                utg = stats_pool.tile([3, 2 * XC], mm.float32, tag="utg")
                for nn in range(2):
                    nc.vector.tensor_copy(utg[:, XC * nn:XC * (nn + 1)],
                                          psT[nn][0:3, :])
                # move rows 1,2 to partition-0 tiles (DVE reads must
                # start at partition 0; DMA has no such limit)
                gw_row = stats_pool.tile([1, 2 * XC], mm.float32, tag="gwr")
                nc.sync.dma_start(out=gw_row[:], in_=utg[1:2, :])
                rs_row = stats_pool.tile([1, 2 * XC], mm.float32, tag="rsr")
                nc.sync.dma_start(out=rs_row[:], in_=utg[2:3, :])
                # cr = sum u_t * rsM'' ; n2 = sum u_t^2 ; dg = sum gw
                scr1 = scr_pool.tile([1, 2 * XC], mm.float32, tag="s1")
                nc.vector.scalar_tensor_tensor(
                    out=scr1[:], in0=utg[0:1, :], scalar=1.0,
                    in1=rs_row[:], op0=ALU.mult, op1=ALU.mult,
                    accum_out=osb[:, 4:5])
                scr2 = scr_pool.tile([1, 2 * XC], mm.float32, tag="s1")
                nc.vector.scalar_tensor_tensor(
                    out=scr2[:], in0=utg[0:1, :], scalar=1.0,
                    in1=utg[0:1, :], op0=ALU.mult, op1=ALU.mult,
                    accum_out=osb[:, 5:6])
                scr3 = scr_pool.tile([1, 2 * XC], mm.float32, tag="s1")
                nc.vector.tensor_scalar(
                    out=scr3[:], in0=gw_row[:], scalar1=1.0, scalar2=0.0,
                    op0=ALU.mult, op1=ALU.add, accum_out=osb[:, 6:7])
                # v2 = sum (0.5*uth + B/2)^2
                upv = stats_pool.tile([1, XC], mm.float32, tag="upv")
                nc.vector.tensor_scalar(
                    out=upv[:], in0=psUP[0:1, :], scalar1=0.5,
                    scalar2=B / 2.0, op0=ALU.mult, op1=ALU.add)
                scr4 = scr_pool.tile([1, XC], mm.float32, tag="s4")
                nc.vector.scalar_tensor_tensor(
                    out=scr4[:], in0=upv[:], scalar=1.0, in1=upv[:],
                    op0=ALU.mult, op1=ALU.mult,
                    accum_out=osb[:, 7:8])

                # ---- partition-spread stats -> stats2 -> psF ----
                scrf = scr_pool.tile([128, NB], mm.float32, tag="sf")
                nc.vector.tensor_scalar(
                    out=scrf[:], in0=fst[:], scalar1=1.0, scalar2=0.0,
                    op0=ALU.mult, op1=ALU.add, accum_out=stats2[:, 0:1])
                scrf2 = scr_pool.tile([128, NB], mm.float32, tag="sf")
                nc.vector.tensor_scalar(
                    out=scrf2[:], in0=f2st[:], scalar1=1.0, scalar2=0.0,
                    op0=ALU.mult, op1=ALU.add, accum_out=stats2[:, 1:2])
                scrm2 = scr_pool.tile([128, 8], mm.float32, tag="sf")
                nc.vector.tensor_scalar(
                    out=scrm2[:], in0=m2st[:], scalar1=1.0, scalar2=0.0,
                    op0=ALU.mult, op1=ALU.add, accum_out=stats2[:, 3:4])

                psF = ps_pool.tile([1, 4], mm.float32, tag="bank")
                nc.tensor.matmul(psF[:], ones_f32[:], stats2[:],
                                 start=True, stop=True)
                nc.vector.tensor_copy(osb[:, 0:4], psF[:])
                nc.sync.dma_start(out=out_ext[:], in_=osb[:])

            if loop_n is None:
                emit_body()
            else:
                with tc.For_i(0, loop_n, 1):
                    emit_body()

    nc.compile()
    return nc


def shard_inputs(inputs: np.ndarray, targets: np.ndarray):
    in_maps = []
    x16 = inputs.astype(ml_dtypes.bfloat16)
    t8 = targets.astype(ml_dtypes.float8_e4m3)
    for c in range(N_CORES):
        r, q = c // 4, c % 4
        mb = 2 * q + r
        ob = 2 * q + (1 - r)
        xq = np.concatenate(
            [x16[:, 256 * mb:256 * (mb + 1)],
             x16[:, 256 * ob:256 * (ob + 1)]], axis=1)
        tblocks = [mb] + [b for b in range(8) if b % 2 == r and b != mb]
        th = np.concatenate(
            [t8[:, 256 * b:256 * (b + 1)] for b in tblocks], axis=1)
        in_maps.append({
            "xq": np.ascontiguousarray(xq),
            "th": np.ascontiguousarray(th),
        })
    return in_maps


def combine_partials(outs) -> np.ndarray:
    """Host-side unshard: combine per-core [1,8] raw slots into the loss."""
    D = float(B) * (B - 1)
    f1 = sum(float(o[0, 0]) for o in outs)
    f2 = sum(float(o[0, 1]) for o in outs)
    p2 = sum(float(o[0, 2]) for o in outs)
    q2 = sum(float(o[0, 3]) for o in outs)
    cr = sum(float(o[0, 4]) for o in outs)
    n2 = sum(float(o[0, 5]) for o in outs)
    dg = sum(float(o[0, 6]) for o in outs)
    v2 = sum(float(o[0, 7]) for o in outs)
    m2 = 0.25 * q2 + 0.5 * cr + 128.0 * n2
    loss = (-ALPHA * (f1 + f2) / (B * L)
            + (0.5 * v2 - 0.5 * p2 - m2 + WS * dg) / D)
    return np.float32(loss)


def kernel(inputs: np.ndarray, targets: np.ndarray) -> np.ndarray:
    if "nc" not in _CACHE:
        _CACHE["nc"] = build_nc()
    nc = _CACHE["nc"]
    in_maps = shard_inputs(np.asarray(inputs), np.asarray(targets))
    res = run_bass_kernel_spmd(nc, in_maps, list(range(N_CORES)))
    return combine_partials([res.results[c]["out"] for c in range(N_CORES)])


if __name__ == "__main__":
    rng = np.random.default_rng(0)
    x = rng.standard_normal((B, L)).astype(np.float32)
    t = (rng.random((B, L)) < 0.25).astype(np.float32)
    got = kernel(x, t)
    print("kernel out:", got)
